# revision 10
# baseline (speedup 1.0000x reference)
"""Multi-head self-attention with KV-cache append, sharded over 8 NeuronCores.

Sharding: batch x heads. Core c handles batch b = c//4 and 4 heads
hb = (c%4)*4 .. hb+4 (two pairs of 2 heads). Each core computes q/k/v
projections for its heads, attention, and a partial output projection
(columns of Wout for its heads); the host sums the 4 partial outputs per
batch and reassembles the k/v cache outputs.

On-device layout (per core):
  xT   [DIM, S]     x transposed (channels on partitions)
  qT   [128, pair, S]    per pair: 2 heads x 64 dims on partitions
  kT   [128, pair, T]    cols 0..PAST-1 from past_keys, rest computed
  v    [128, tc, 256]    natural [t, d] layout, 20 t-chunks, 4 heads of cols
  PT   [128, tc, 512]    exp(scores^T) per head: t on partitions, s free
  attnT[128, pair, S]    unnormalized attn output^T, normalized via 1/denom
Scores are computed transposed (t on partitions) so P^T feeds attn@V
directly; softmax denominators come from an extra ones-vector matmul
accumulated alongside attn@V; no max-subtraction (scores are O(1) by
construction: softmax is shift-invariant and inputs are ~N(0,1)-scaled).
All matmuls run in float32r (full fp32 storage; PE rounds operands to
~tf32 precision at consume, fp32 accumulate).
"""

import numpy as np

import concourse.bass as bass
import concourse.mybir as mybir
import concourse.tile as tile
from concourse.bass_utils import run_bass_kernel_spmd

B, S, PAST, DIM, H, Dh = 2, 2048, 512, 1024, 16, 64
T = S + PAST            # 2560
N_CORES = 8
HPC = 4                 # heads per core
NPAIR = 2               # head pairs per core
SC = 512                # s-chunk (psum bank width in fp32)
NSC = S // SC           # 4
TCH = T // 128          # 20 t-chunks
PCH = PAST // 128       # 4 past t-chunks
CCH = DIM // 128        # 8 contraction chunks
G = 3                   # t-chunks per exp group (3+3 score banks + 2 pv banks = 8)
VA = Dh + 1             # v columns per head incl. fused ones column (denominator)
F32 = mybir.dt.float32
F32R = mybir.dt.float32r
SCALE = 1.0 / np.sqrt(np.float32(DIM))

_CACHED_NC = None


def _split_multi_waits(nc):
    # This walrus build rejects >1 sync-wait command on several instruction
    # structs; hoist all but the last wait onto standalone event-semaphore
    # instructions directly before the offender, same engine.
    n = 0
    for fn in nc.m.functions:
        for bb in fn.blocks:
            out = []
            changed = False
            for inst in bb.instructions:
                si = inst.sync_info
                if si is not None and len(si.on_wait) > 1:
                    waits = list(si.on_wait)
                    for w in waits[:-1]:
                        ev = mybir.InstEventSemaphore(
                            name=f"I-waitfix-{n}", engine=inst.engine, ins=[], outs=[]
                        )
                        ev.sync_info = mybir.SyncInfo(on_wait=[w], on_update=[])
                        nc.register_instruction(ev)
                        out.append(ev)
                        n += 1
                    inst.sync_info = mybir.SyncInfo(
                        on_wait=[waits[-1]], on_update=list(si.on_update)
                    )
                    changed = True
                out.append(inst)
            if changed:
                bb.instructions = out
    return n


def _build():
    nc = bass.Bass()
    xT = nc.dram_tensor("xT", [DIM, S], F32, kind="ExternalInput")
    wqT = nc.dram_tensor("wqT", [DIM, HPC * Dh], F32, kind="ExternalInput")
    wkT = nc.dram_tensor("wkT", [DIM, HPC * Dh], F32, kind="ExternalInput")
    wvT = nc.dram_tensor("wvT", [DIM, HPC * Dh], F32, kind="ExternalInput")
    woT = nc.dram_tensor("woT", [HPC * Dh, DIM], F32, kind="ExternalInput")
    pkT = nc.dram_tensor("pkT", [NPAIR, 128, PAST], F32, kind="ExternalInput")
    pv = nc.dram_tensor("pv", [PAST, HPC * Dh], F32, kind="ExternalInput")
    bq = nc.dram_tensor("bq", [128, NPAIR], F32, kind="ExternalInput")
    bk = nc.dram_tensor("bk", [128, NPAIR], F32, kind="ExternalInput")
    bv = nc.dram_tensor("bv", [1, HPC * Dh], F32, kind="ExternalInput")
    ones_d = nc.dram_tensor("ones_d", [128, TCH * HPC], F32, kind="ExternalInput")
    outp = nc.dram_tensor("outp", [S, DIM], F32, kind="ExternalOutput")
    knew = nc.dram_tensor("knew", [NPAIR, 128, S], F32, kind="ExternalOutput")
    vnew = nc.dram_tensor("vnew", [S // 128, 128, HPC * Dh], F32, kind="ExternalOutput")

    with tile.TileContext(nc) as tc:
        with (
            tc.tile_pool(name="persist", bufs=1) as pp,
            tc.tile_pool(name="small", bufs=1) as sp,
        ):
            wq = pp.tile([128, CCH, HPC * Dh], F32R)
            wk = pp.tile([128, CCH, HPC * Dh], F32R)
            wv = pp.tile([128, CCH, HPC * Dh], F32R)
            wo = pp.tile([128, NPAIR, DIM], F32R)
            qT = pp.tile([128, NPAIR, S], F32R)
            kT = pp.tile([128, NPAIR, T], F32R)
            vt = pp.tile([128, TCH, HPC * VA], F32R)
            attnT = pp.tile([128, NPAIR, S], F32R)
            bq_sb = sp.tile([128, NPAIR], F32)
            bk_sb = sp.tile([128, NPAIR], F32)
            bv_sb = sp.tile([128, HPC * Dh], F32)

            # ones columns of v (one per head, used for softmax denominators)
            nc.sync.dma_start(
                out=vt[:].rearrange("p t (h e) -> p t h e", e=VA)[:, :, :, Dh : Dh + 1],
                in_=ones_d[:].rearrange("p (t h) -> p t h", h=HPC).bitcast(F32R)[
                    :, :, :, None
                ],
            )
            nc.sync.dma_start(out=bq_sb[:], in_=bq[:])
            nc.sync.dma_start(out=bk_sb[:], in_=bk[:])
            nc.sync.dma_start(out=bv_sb[:], in_=bv[:].to_broadcast((128, HPC * Dh)))
            nc.sync.dma_start(
                out=wq[:], in_=wqT[:].rearrange("(c p) d -> p c d", p=128).bitcast(F32R)
            )
            nc.sync.dma_start(
                out=wk[:], in_=wkT[:].rearrange("(c p) d -> p c d", p=128).bitcast(F32R)
            )
            nc.sync.dma_start(
                out=wv[:], in_=wvT[:].rearrange("(c p) d -> p c d", p=128).bitcast(F32R)
            )
            nc.sync.dma_start(
                out=wo[:], in_=woT[:].rearrange("(j p) o -> p j o", p=128).bitcast(F32R)
            )
            for pair in range(NPAIR):
                nc.sync.dma_start(out=kT[:, pair, 0:PAST], in_=pkT[pair].bitcast(F32R))
            for c in range(PCH):
                nc.sync.dma_start(
                    out=vt[:, c, :].rearrange("p (h e) -> p h e", e=VA)[:, :, 0:Dh],
                    in_=pv[c * 128 : (c + 1) * 128, :]
                    .rearrange("p (h d) -> p h d", d=Dh)
                    .bitcast(F32R),
                )

            # ---- phase 1: qkv projections ----
            with (
                tc.tile_pool(name="xp", bufs=1) as xp,
                tc.tile_pool(name="ps1", bufs=1, space="PSUM") as ps1,
                tc.tile_pool(name="ps1v", bufs=2, space="PSUM") as ps1v,
            ):
                xt = xp.tile([128, CCH, S], F32R)
                for cc in range(CCH):
                    nc.sync.dma_start(
                        out=xt[:, cc, :],
                        in_=xT[cc * 128 : (cc + 1) * 128, :].bitcast(F32R),
                    )
                # qT / kT-new: out [d-pair, s], weights stationary (reused over sc)
                for tgt, w, bias, off in ((qT, wq, bq_sb, None), (kT, wk, bk_sb, PAST)):
                    for pair in range(NPAIR):
                        psq = ps1.tile([128, NSC, SC], F32, tag="qk")
                        for cc in range(CCH):
                            for sc in range(NSC):
                                nc.tensor.matmul(
                                    psq[:, sc, :],
                                    w[:, cc, pair * 128 : (pair + 1) * 128],
                                    xt[:, cc, sc * SC : (sc + 1) * SC],
                                    start=(cc == 0),
                                    stop=(cc == CCH - 1),
                                )
                        for sc in range(NSC):
                            lo = sc * SC if off is None else off + sc * SC
                            nc.vector.tensor_scalar_add(
                                tgt[:, pair, lo : lo + SC],
                                psq[:, sc, :],
                                bias[:, pair : pair + 1],
                            )
                # v natural [s, d]: x stationary per (st, cc)
                for st in range(S // 128):
                    psv = ps1v.tile([128, HPC * Dh], F32, tag="v")
                    for cc in range(CCH):
                        nc.tensor.matmul(
                            psv[:],
                            xt[:, cc, st * 128 : (st + 1) * 128],
                            wv[:, cc, :],
                            start=(cc == 0),
                            stop=(cc == CCH - 1),
                        )
                    nc.vector.tensor_tensor(
                        vt[:, PCH + st, :].rearrange("p (h e) -> p h e", e=VA)[
                            :, :, 0:Dh
                        ],
                        psv[:].rearrange("p (h d) -> p h d", d=Dh),
                        bv_sb[:].rearrange("p (h d) -> p h d", d=Dh),
                        mybir.AluOpType.add,
                    )

            # outputs that only depend on phase 1
            for pair in range(NPAIR):
                nc.sync.dma_start(
                    out=knew[pair], in_=kT[:, pair, PAST:T].bitcast(F32)
                )
            for st in range(S // 128):
                nc.sync.dma_start(
                    out=vnew[st].rearrange("p (h d) -> p h d", d=Dh),
                    in_=vt[:, PCH + st, :]
                    .rearrange("p (h e) -> p h e", e=VA)[:, :, 0:Dh]
                    .bitcast(F32),
                )

            # ---- phase 2: attention ----
            with (
                tc.tile_pool(name="ptp", bufs=1) as ptp,
                tc.tile_pool(name="rp", bufs=2) as rp,
                tc.tile_pool(name="rdram", bufs=2, space="DRAM") as rdp,
                tc.tile_pool(name="psS", bufs=1, space="PSUM") as psS,
                tc.tile_pool(name="psP", bufs=1, space="PSUM") as psP,
                tc.tile_pool(name="psD", bufs=1, space="PSUM") as psD,
            ):
                for sc in range(NSC):
                    for pair in range(NPAIR):
                        pt0 = ptp.tile([128, TCH, SC], F32R, tag="pt0")
                        pt1 = ptp.tile([128, TCH, SC], F32R, tag="pt1")
                        g0 = 0
                        while g0 < TCH:
                            gn = min(G, TCH - g0)
                            ss0 = psS.tile([128, G, SC], F32, tag="s0")
                            ss1 = psS.tile([128, G, SC], F32, tag="s1")
                            for i in range(gn):
                                tt = g0 + i
                                nc.tensor.matmul(
                                    ss0[:, i, :],
                                    kT[0:64, pair, tt * 128 : (tt + 1) * 128],
                                    qT[0:64, pair, sc * SC : (sc + 1) * SC],
                                    start=True,
                                    stop=True,
                                    tile_position=(0, 0),
                                )
                                nc.tensor.matmul(
                                    ss1[:, i, :],
                                    kT[64:128, pair, tt * 128 : (tt + 1) * 128],
                                    qT[64:128, pair, sc * SC : (sc + 1) * SC],
                                    start=True,
                                    stop=True,
                                    tile_position=(64, 0),
                                )
                            nc.scalar.activation(
                                pt0[:, g0 : g0 + gn, :],
                                ss0[:, 0:gn, :],
                                mybir.ActivationFunctionType.Exp,
                                scale=float(SCALE),
                            )
                            nc.scalar.activation(
                                pt1[:, g0 : g0 + gn, :],
                                ss1[:, 0:gn, :],
                                mybir.ActivationFunctionType.Exp,
                                scale=float(SCALE),
                            )
                            g0 += gn
                        # attn @ V per head, M=65: rows 0..63 = attn-out^T,
                        # row 64 = softmax denominator (fused ones column of v)
                        pvA = psP.tile([65, SC], F32, tag="pvA")
                        pvB = psD.tile([65, SC], F32, tag="pvB")
                        for tt in range(TCH):
                            st_flags = dict(start=(tt == 0), stop=(tt == TCH - 1))
                            nc.tensor.matmul(
                                pvA[:],
                                vt[:, tt, (2 * pair) * VA : (2 * pair + 1) * VA],
                                pt0[:, tt, :],
                                **st_flags,
                            )
                            nc.tensor.matmul(
                                pvB[:],
                                vt[:, tt, (2 * pair + 1) * VA : (2 * pair + 2) * VA],
                                pt1[:, tt, :],
                                **st_flags,
                            )
                        rA = rp.tile([65, SC], F32, tag="rA")
                        rB = rp.tile([65, SC], F32, tag="rB")
                        nc.vector.reciprocal(rA[64:65, :], pvA[64:65, :])
                        nc.vector.reciprocal(rB[64:65, :], pvB[64:65, :])
                        # free-dim broadcast of 1/denom: bounce rows through
                        # DRAM (engines can't replicate across partitions)
                        r_dram = rdp.tile([2, SC], F32, tag="rd")
                        nc.sync.dma_start(out=r_dram[0:1, :], in_=rA[64:65, :])
                        nc.sync.dma_start(out=r_dram[1:2, :], in_=rB[64:65, :])
                        rbc0 = rp.tile([64, SC], F32, tag="rbc0")
                        rbc1 = rp.tile([64, SC], F32, tag="rbc1")
                        nc.sync.dma_start(
                            out=rbc0[:], in_=r_dram[0:1, :].to_broadcast((64, SC))
                        )
                        nc.sync.dma_start(
                            out=rbc1[:], in_=r_dram[1:2, :].to_broadcast((64, SC))
                        )
                        # head0 -> attnT partitions 0..63 directly; head1 is
                        # normalized into a staging tile then DMA-shifted to
                        # partitions 64..127 (engines cannot cross partitions)
                        nc.vector.tensor_tensor(
                            attnT[0:64, pair, sc * SC : (sc + 1) * SC],
                            pvA[0:64, :],
                            rbc0[:],
                            mybir.AluOpType.mult,
                        )
                        tmpB = rp.tile([64, SC], F32R, tag="tmpB")
                        nc.vector.tensor_tensor(
                            tmpB[:],
                            pvB[0:64, :],
                            rbc1[:],
                            mybir.AluOpType.mult,
                        )
                        nc.sync.dma_start(
                            out=attnT[64:128, pair, sc * SC : (sc + 1) * SC],
                            in_=tmpB[:],
                        )

            # ---- phase 3: output projection (partial over this core's heads) ----
            with (
                tc.tile_pool(name="op", bufs=3) as op,
                tc.tile_pool(name="psO", bufs=2, space="PSUM") as psO,
            ):
                for st in range(S // 128):
                    for oc in range(DIM // SC):
                        pso = psO.tile([128, SC], F32, tag="o")
                        for pair in range(NPAIR):
                            nc.tensor.matmul(
                                pso[:],
                                attnT[:, pair, st * 128 : (st + 1) * 128],
                                wo[:, pair, oc * SC : (oc + 1) * SC],
                                start=(pair == 0),
                                stop=(pair == NPAIR - 1),
                            )
                        ob = op.tile([128, SC], F32, tag="ob")
                        nc.vector.tensor_copy(ob[:], pso[:])
                        nc.sync.dma_start(
                            out=outp[st * 128 : (st + 1) * 128, oc * SC : (oc + 1) * SC],
                            in_=ob[:],
                        )

    _split_multi_waits(nc)
    nc.finalize()
    return nc


def _get_nc():
    global _CACHED_NC
    if _CACHED_NC is None:
        _CACHED_NC = _build()
    return _CACHED_NC


def _build_in_maps(inputs):
    x = np.asarray(inputs["x"], dtype=np.float32)
    past_keys = np.asarray(inputs["past_keys"], dtype=np.float32)
    past_values = np.asarray(inputs["past_values"], dtype=np.float32)
    Wqkv = np.asarray(inputs["Wqkv"], dtype=np.float32)
    bqkv = np.asarray(inputs["bqkv"], dtype=np.float32)
    Wout = np.asarray(inputs["Wout"], dtype=np.float32)

    in_maps = []
    for c in range(N_CORES):
        b = c // 4
        hb = (c % 4) * HPC
        rows = slice(hb * Dh, (hb + HPC) * Dh)
        wq_s = Wqkv[rows, :]                      # [256, DIM]
        wk_s = Wqkv[DIM + rows.start : DIM + rows.stop, :]
        wv_s = Wqkv[2 * DIM + rows.start : 2 * DIM + rows.stop, :]
        bq_s = bqkv[rows]
        bk_s = bqkv[DIM + rows.start : DIM + rows.stop]
        bv_s = bqkv[2 * DIM + rows.start : 2 * DIM + rows.stop]
        in_maps.append(
            {
                "xT": np.ascontiguousarray(x[b].T),
                "wqT": np.ascontiguousarray(wq_s.T),
                "wkT": np.ascontiguousarray(wk_s.T),
                "wvT": np.ascontiguousarray(wv_s.T),
                "woT": np.ascontiguousarray(Wout[:, rows].T),
                "pkT": np.ascontiguousarray(
                    past_keys[b, hb : hb + HPC].transpose(0, 2, 1)
                ).reshape(NPAIR, 128, PAST),
                "pv": np.ascontiguousarray(
                    past_values[b, hb : hb + HPC].transpose(1, 0, 2)
                ).reshape(PAST, HPC * Dh),
                "bq": np.ascontiguousarray(bq_s.reshape(NPAIR, 128).T),
                "bk": np.ascontiguousarray(bk_s.reshape(NPAIR, 128).T),
                "bv": bv_s.reshape(1, HPC * Dh).copy(),
                "ones_d": np.ones((128, TCH * HPC), dtype=np.float32),
            }
        )
    return in_maps


def kernel(x, mask, past_keys, past_values, Wqkv, bqkv, Wout, bout):
    past_keys = np.asarray(past_keys, dtype=np.float32)
    past_values = np.asarray(past_values, dtype=np.float32)
    bout = np.asarray(bout, dtype=np.float32)
    nc = _get_nc()
    in_maps = _build_in_maps(
        dict(x=x, past_keys=past_keys, past_values=past_values, Wqkv=Wqkv,
             bqkv=bqkv, Wout=Wout)
    )
    res = run_bass_kernel_spmd(nc, in_maps, list(range(N_CORES)))

    out = np.zeros((B, S, DIM), dtype=np.float32)
    k = np.empty((B, H, T, Dh), dtype=np.float32)
    v = np.empty((B, H, T, Dh), dtype=np.float32)
    k[:, :, :PAST, :] = past_keys
    v[:, :, :PAST, :] = past_values
    for c in range(N_CORES):
        b = c // 4
        hb = (c % 4) * HPC
        r = res.results[c]
        out[b] += r["outp"]
        # knew [pair, 128, S] -> [4 heads, 64, S] -> [4, S, 64]
        k[b, hb : hb + HPC, PAST:, :] = (
            r["knew"].reshape(HPC, Dh, S).transpose(0, 2, 1)
        )
        # vnew [S//128, 128, 256] -> [S, 4, 64] -> [4, S, 64]
        v[b, hb : hb + HPC, PAST:, :] = (
            r["vnew"].reshape(S, HPC, Dh).transpose(1, 0, 2)
        )
    out += bout[None, None, :]
    return out, k, v
